# revision 38
# baseline (speedup 1.0000x reference)
"""Multi-head attention (b=8, c=512, t=1024, 8 heads, e=64) on 8 TRN2 cores.

Strategy: pure data-parallel over batch — each NeuronCore handles one batch
element; weights are replicated; no collectives.

Per-core math (all matmuls in bf16, accumulation f32 in PSUM):
  x           [512, 1024]  (c-major, t = h*w tokens)
  q,k = W x   channel-major tiles [128, 1024] (o-tiles 0-3 = q with the 1/8
              softmax scale folded into Wq/bq on host, 4-7 = k)
  vT  = xT Wv token-major tiles [128, 512] (+v bias via K=1 ones-matmul;
        valid because softmax rows sum to 1 only post-normalisation)
  scoresT_h [T, t] = k_h^T q_h  (K=64, head pair row-packed: A rows 0-63,
        B rows 64-127 of the PE array; outputs share one [128,1024] PSUM
        tile: A in cols 0-511, B in cols 512-1023 — one exp per chunk)
  P = exp(scoresT)  (softmax max-subtraction skipped: scores ~ N(0,1))
  l_h[t] = ones^T P (K=128 M=1 matmul, col-packed pair)
  avT_h [e, t] = v_h^T P (col-packed head pairs), normalise by 1/l via
        K=1 broadcast matmul + reciprocal_approx, fold into bf16 attnout
  y = Wproj attnout + b  -> bf16 out [512, 1024] (cast f32 on host)

Two outer passes over t-halves (th); emission is software-pipelined: per
chunk u emit scores+exp(u) then av/l(u-1), weaving qk/v/proj GEMMs into
chunk slots as PE filler so the exp stream never starves.
"""

import numpy as np
import ml_dtypes

_CACHE = {}

B, C, T = 8, 512, 1024
NH, E = 8, 64


def _build():
    import concourse.tile as tile
    from concourse import bacc, mybir

    F32 = mybir.dt.float32
    BF16 = mybir.dt.bfloat16
    AF = mybir.ActivationFunctionType
    ALU = mybir.AluOpType

    import concourse.bass as bass

    nc = bacc.Bacc()

    x_d = nc.declare_dram_parameter("x", [C, T], BF16, isOutput=False)
    wqkv_d = nc.declare_dram_parameter("wqkv", [C, 3 * C], BF16, isOutput=False)
    wproj_d = nc.declare_dram_parameter("wproj", [C, C], BF16, isOutput=False)
    bqk_d = nc.declare_dram_parameter("bqk", [128, 8], F32, isOutput=False)
    bvp_d = nc.declare_dram_parameter("bvp", [128, 4], F32, isOutput=False)
    bproj_d = nc.declare_dram_parameter("bproj", [128, 4], F32, isOutput=False)
    out_d = nc.declare_dram_parameter("out", [C, T], BF16, isOutput=True)

    with tile.TileContext(nc) as tc:
        with (
            tc.tile_pool(name="persist", bufs=1) as per,
            tc.tile_pool(name="ppool", bufs=2) as pp,
            tc.tile_pool(name="scr", bufs=3) as scr,
            tc.tile_pool(name="dram", bufs=8, space="DRAM") as dr,
            tc.tile_pool(name="psc", bufs=2, space="PSUM") as psc,
            tc.tile_pool(name="pav", bufs=2, space="PSUM") as pav,
            tc.tile_pool(name="pms", bufs=2, space="PSUM") as pms,
        ):
            # ---- ones + PE warmup first (no input deps): release the HAM
            # clock gate while input DMAs land.
            ones_all = per.tile([128, 512], BF16, tag="ones_all")
            nc.vector.memset(ones_all[:], 1.0)
            pswarm = pms.tile([128, 512], F32, tag="ms", name="pswarm")
            for i in range(12):
                nc.tensor.matmul(
                    pswarm[:], lhsT=ones_all[:, 0:128], rhs=ones_all[:],
                    start=True, stop=True,
                )

            # ---- input DMAs. Critical path: x + q-part + k-part of wqkv +
            # bqk gate the first GEMMs — spread across engine queues.
            bqk = per.tile([128, 8], F32, tag="bqk")
            nc.scalar.dma_start(out=bqk[:], in_=bqk_d[:, :])
            xs = []
            for c in range(4):
                xt = per.tile([128, T], BF16, tag=f"x{c}", name=f"x{c}")
                eng = nc.sync if c < 2 else nc.scalar
                eng.dma_start(out=xt[:], in_=x_d[128 * c : 128 * (c + 1), :])
                xs.append(xt)
            wqkv = [
                per.tile([128, 3 * C], BF16, tag=f"wqkv{c}", name=f"wqkv{c}")
                for c in range(4)
            ]
            for c in range(4):  # q columns (gate qk_gemm(0..3))
                nc.gpsimd.dma_start(
                    out=wqkv[c][:, 0:C], in_=wqkv_d[128 * c : 128 * (c + 1), 0:C]
                )
            for c in range(4):  # k columns
                nc.scalar.dma_start(
                    out=wqkv[c][:, C : 2 * C],
                    in_=wqkv_d[128 * c : 128 * (c + 1), C : 2 * C],
                )
            bvp = per.tile([128, 4], F32, tag="bvp")
            nc.gpsimd.dma_start(out=bvp[:], in_=bvp_d[:, :])
            for c in range(4):  # v columns
                nc.sync.dma_start(
                    out=wqkv[c][:, 2 * C : 3 * C],
                    in_=wqkv_d[128 * c : 128 * (c + 1), 2 * C : 3 * C],
                )
            wproj = []
            for c in range(4):
                w = per.tile([128, C], BF16, tag=f"wproj{c}", name=f"wproj{c}")
                nc.gpsimd.dma_start(out=w[:], in_=wproj_d[128 * c : 128 * (c + 1), :])
                wproj.append(w)
            bproj = per.tile([128, 4], F32, tag="bproj")
            nc.gpsimd.dma_start(out=bproj[:], in_=bproj_d[:, :])

            # ---- persistent activations ----
            qk = [per.tile([128, T], BF16, tag=f"qk{o}", name=f"qk{o}") for o in range(8)]
            vT = [per.tile([128, C], BF16, tag=f"vT{t}", name=f"vT{t}") for t in range(8)]
            attnout = [
                per.tile([128, T], BF16, tag=f"ao{p}", name=f"ao{p}") for p in range(4)
            ]
            out_sb = [
                per.tile([128, T], BF16, tag=f"os{o}", name=f"os{o}") for o in range(4)
            ]

            qk_ps = {}

            def qk_gemm_half(o, half):
                # half 0/1 = t-columns [0:512] / [512:1024] of the qk GEMM
                if half == 0:
                    qk_ps[o] = psc.tile([128, T], F32, tag="sc", name=f"psqk{o}")
                ps = qk_ps[o]
                for c in range(4):
                    nc.tensor.matmul(
                        ps[:, 512 * half : 512 * (half + 1)],
                        lhsT=wqkv[c][:, 128 * o : 128 * (o + 1)],
                        rhs=xs[c][:, 512 * half : 512 * (half + 1)],
                        start=(c == 0),
                        stop=(c == 3),
                    )
                if half == 1:
                    if o == 0:  # ScalarE is idle pre-attention; Identity is in
                        # the exp table set (no ACT_TABLE_LOAD switch)
                        nc.scalar.activation(
                            qk[o][:], ps[:], AF.Identity, bias=bqk[:, o : o + 1]
                        )
                    else:
                        nc.vector.tensor_scalar_add(qk[o][:], ps[:], bqk[:, o : o + 1])

            v_ps = {}

            def v_gemm_half(tt, half):
                if half == 0:
                    v_ps[tt] = pms.tile([128, 512], F32, tag="ms", name=f"psv{tt}")
                    for c in range(2):
                        nc.tensor.matmul(
                            v_ps[tt][:],
                            lhsT=xs[c][:, 128 * tt : 128 * (tt + 1)],
                            rhs=wqkv[c][:, 2 * C : 3 * C],
                            start=(c == 0), stop=False,
                        )
                else:
                    ps = v_ps[tt]
                    for c in range(2, 4):
                        nc.tensor.matmul(
                            ps[:],
                            lhsT=xs[c][:, 128 * tt : 128 * (tt + 1)],
                            rhs=wqkv[c][:, 2 * C : 3 * C],
                            start=False, stop=(c == 3),
                        )
                    nc.vector.tensor_copy(vT[tt][:], ps[:])

            def scexp(p, th, k):
                tsl = slice(512 * th, 512 * (th + 1))
                ksl = slice(128 * k, 128 * (k + 1))
                ps = psc.tile([128, T], F32, tag="sc", name=f"pssc{p}_{th}_{k}")
                nc.tensor.matmul(
                    ps[:, 0:512],
                    lhsT=qk[4 + p][0:64, ksl], rhs=qk[p][0:64, tsl],
                    start=True, stop=True, tile_position=(0, 0),
                )
                nc.tensor.matmul(
                    ps[:, 512:1024],
                    lhsT=qk[4 + p][64:128, ksl], rhs=qk[p][64:128, tsl],
                    start=True, stop=True, tile_position=(64, 0),
                )
                P = pp.tile([128, T], BF16, tag=f"P{k}", bufs=3,
                            name=f"P{p}_{th}_{k}")
                nc.scalar.activation(P[:], ps[:], AF.Exp)
                return P

            def av_mms(p, th, k, P, psav, psl):
                hA, hB = 2 * p, 2 * p + 1
                nc.tensor.matmul(
                    psav[0:64, :],
                    lhsT=vT[k][:, 64 * hA : 64 * (hA + 1)], rhs=P[:, 0:512],
                    start=(k == 0), stop=(k == 7), tile_position=(0, 0),
                )
                nc.tensor.matmul(
                    psav[64:128, :],
                    lhsT=vT[k][:, 64 * hB : 64 * (hB + 1)], rhs=P[:, 512:1024],
                    start=(k == 0), stop=(k == 7), tile_position=(0, 64),
                )
                nc.tensor.matmul(
                    psl[0:1, :],
                    lhsT=ones_all[:, 0:1], rhs=P[:, 0:512],
                    start=(k == 0), stop=(k == 7), tile_position=(0, 0),
                )
                nc.tensor.matmul(
                    psl[32:33, :],
                    lhsT=ones_all[:, 0:1], rhs=P[:, 512:1024],
                    start=(k == 0), stop=(k == 7), tile_position=(0, 32),
                )

            def finalize(p, th, psav, psl):
                tsl = slice(512 * th, 512 * (th + 1))
                lb = scr.tile([128, 512], BF16, tag="lb", name=f"lb{p}_{th}")
                nc.vector.tensor_copy(lb[:], psl[:])
                psbc = pms.tile([128, 512], F32, tag="ms", name=f"psbc{p}_{th}")
                nc.tensor.matmul(
                    psbc[0:64, :], lhsT=ones_all[0:1, 0:64], rhs=lb[0:1, :],
                    start=True, stop=True, tile_position=(0, 0),
                )
                nc.tensor.matmul(
                    psbc[64:128, :], lhsT=ones_all[32:33, 0:64], rhs=lb[32:33, :],
                    start=True, stop=True, tile_position=(32, 64),
                )
                bc = scr.tile([128, 512], F32, tag="bc", name=f"bc{p}_{th}")
                nc.vector.reciprocal_approx_fast(bc[:], psbc[:])
                tmpn = scr.tile([128, 512], F32, tag="tmpn", name=f"tn{p}_{th}")
                nc.vector.tensor_mul(tmpn[:], psav[:], bc[:])
                nc.vector.tensor_scalar_add(
                    attnout[p][:, tsl], tmpn[:], bvp[:, p : p + 1]
                )

            def proj(o, th):
                tsl = slice(512 * th, 512 * (th + 1))
                ps = pms.tile([128, 512], F32, tag="ms", name=f"pspr{o}_{th}")
                for c in range(4):
                    nc.tensor.matmul(
                        ps[:],
                        lhsT=wproj[c][:, 128 * o : 128 * (o + 1)],
                        rhs=attnout[c][:, tsl],
                        start=(c == 0),
                        stop=(c == 3),
                    )
                nc.vector.tensor_scalar_add(out_sb[o][:, tsl], ps[:], bproj[:, o : o + 1])
                eng = nc.sync if o % 2 == 0 else nc.gpsimd
                eng.dma_start(
                    out=out_d[128 * o : 128 * (o + 1), tsl], in_=out_sb[o][:, tsl]
                )

            # ---- schedule: two passes over t-halves, software-pipelined;
            # av/l batched two chunks behind the score/exp stream.
            pending = []
            fin_q = []
            carry = {}

            def emit_avl(limit=1):
                nonlocal pending
                if len(pending) <= limit:
                    return
                todo, pending = pending[:-limit] if limit else pending, \
                    pending[-limit:] if limit else []
                for u in todo:
                    av_mms(*u)

            def pe_keepwarm(n=3):
                for _ in range(n):
                    nc.tensor.ldweights(ones_all[:, 0:128])

            quarters = [
                (0, (0, 1)), (1, (0, 1)), (0, (2, 3)), (1, (2, 3)),
            ]
            for qi, (th, pairs) in enumerate(quarters):
                fill = {}
                if qi == 0:
                    # v GEMM halves through pair 0; qk1/qk5 late in pair 0
                    fill[(0, 0)] = [lambda: v_gemm_half(0, 0), lambda: v_gemm_half(0, 1),
                                    lambda: v_gemm_half(1, 0)]
                    fill[(0, 1)] = [lambda: v_gemm_half(1, 1), lambda: v_gemm_half(2, 0)]
                    fill[(0, 2)] = [lambda: v_gemm_half(2, 1), lambda: v_gemm_half(3, 0)]
                    fill[(0, 3)] = [lambda: v_gemm_half(3, 1), lambda: v_gemm_half(4, 0)]
                    fill[(0, 4)] = [lambda: v_gemm_half(4, 1), lambda: v_gemm_half(5, 0),
                                    lambda: qk_gemm_half(5, 0)]
                    fill[(0, 5)] = [lambda: v_gemm_half(5, 1), lambda: v_gemm_half(6, 0),
                                    lambda: qk_gemm_half(5, 1)]
                    fill[(0, 6)] = [lambda: v_gemm_half(6, 1), lambda: v_gemm_half(7, 0),
                                    lambda: qk_gemm_half(1, 0)]
                    fill[(0, 7)] = [lambda: v_gemm_half(7, 1), lambda: qk_gemm_half(1, 1)]
                elif qi == 1:
                    fill[(0, 1)] = [lambda: qk_gemm_half(2, 0)]
                    fill[(0, 4)] = [lambda: qk_gemm_half(2, 1)]
                    fill[(1, 1)] = [lambda: qk_gemm_half(6, 0)]
                    fill[(1, 4)] = [lambda: qk_gemm_half(6, 1)]
                elif qi == 2:
                    fill[(2, 0)] = [lambda: qk_gemm_half(3, 0)]
                    fill[(2, 2)] = [lambda: qk_gemm_half(3, 1)]
                    fill[(2, 4)] = [lambda: qk_gemm_half(7, 0)]
                    fill[(2, 6)] = [lambda: qk_gemm_half(7, 1)]
                else:
                    fill[(2, 2)] = [lambda: proj(0, 0)]
                    fill[(2, 4)] = [lambda: proj(1, 0)]
                    fill[(2, 6)] = [lambda: proj(2, 0)]
                    fill[(3, 2)] = [lambda: proj(3, 0)]
                if qi == 0:
                    qk_gemm_half(0, 0)
                    qk_gemm_half(0, 1)
                    qk_gemm_half(4, 0)
                    qk_gemm_half(4, 1)
                for p in pairs:
                    psav = pav.tile([128, 512], F32, tag="av", name=f"psav{p}_{th}")
                    psl = pms.tile([128, 512], F32, tag="ms", name=f"psl{p}_{th}")
                    last = (qi == 3 and p == 3)
                    for k in range(8):
                        pe_keepwarm()
                        P = scexp(p, th, k)
                        if k == 0:
                            emit_avl(limit=2)
                        elif k == 1:
                            emit_avl(limit=0)
                        elif last:
                            emit_avl(limit=1)
                        elif k % 2 == 0:
                            emit_avl(limit=2)
                        for f in fill.get((p, k), []):
                            f()
                        while fin_q:
                            fin_q.pop(0)()
                        pending.append((p, th, k, P, psav, psl))
                        if k == 0 and (p != 0 or th != 0):
                            pp_, pth, pav_, pl_ = carry["prev"]
                            fin_q.append(
                                lambda a=pp_, b=pth, c=pav_, d=pl_: finalize(a, b, c, d)
                            )
                    carry["prev"] = (p, th, psav, psl)
            emit_avl(limit=0)
            finalize(*carry["prev"])
            for o in range(4):
                proj(o, 1)

    nc.compile()
    return nc


def _get_nc():
    if "nc" not in _CACHE:
        _CACHE["nc"] = _build()
    return _CACHE["nc"]


def kernel(x, qkv_w, qkv_b, proj_w, proj_b, _trace=False):
    from concourse.bass_utils import run_bass_kernel_spmd

    nc = _get_nc()

    bf16 = ml_dtypes.bfloat16
    b, c, h, w = x.shape
    xf = np.asarray(x, dtype=np.float32).reshape(b, c, h * w)
    qkv_b = np.asarray(qkv_b, dtype=np.float32)
    qkv_w = np.asarray(qkv_w, dtype=np.float32)
    # fold the 1/sqrt(e)=1/8 softmax scale into Wq / bq on host
    qkv_w = np.concatenate([qkv_w[:512] * 0.125, qkv_w[512:]], axis=0)
    bq = np.concatenate([qkv_b[:512] * 0.125, qkv_b[512:1024]])
    wqkvT = np.ascontiguousarray(qkv_w.T).astype(bf16)
    wprojT = np.ascontiguousarray(np.asarray(proj_w, np.float32).T).astype(bf16)
    bqk = np.ascontiguousarray(bq.reshape(8, 128).T)
    bvp = np.ascontiguousarray(qkv_b[1024:1536].reshape(4, 128).T)
    bproj = np.ascontiguousarray(np.asarray(proj_b, np.float32).reshape(4, 128).T)

    in_maps = [
        dict(
            x=np.ascontiguousarray(xf[i]).astype(bf16),
            wqkv=wqkvT,
            wproj=wprojT,
            bqk=bqk,
            bvp=bvp,
            bproj=bproj,
        )
        for i in range(b)
    ]
    res = run_bass_kernel_spmd(nc, in_maps, core_ids=list(range(8)), trace=_trace)
    out = np.stack([res.results[i]["out"].astype(np.float32) for i in range(b)])
    out = out.reshape(b, c, h, w)
    if _trace:
        _CACHE["last_result"] = res
    return out


# revision 39
# speedup vs baseline: 1.0786x; 1.0786x over previous
"""Multi-head attention (b=8, c=512, t=1024, 8 heads, e=64) on 8 TRN2 cores.

Strategy: pure data-parallel over batch — each NeuronCore handles one batch
element; weights are replicated; no collectives.

Per-core math (all matmuls in bf16, accumulation f32 in PSUM):
  x           [512, 1024]  (c-major, t = h*w tokens)
  q,k = W x   channel-major tiles [128, 1024] (o-tiles 0-3 = q with the 1/8
              softmax scale folded into Wq/bq on host, 4-7 = k)
  vT  = xT Wv token-major tiles [128, 512] (+v bias via K=1 ones-matmul;
        valid because softmax rows sum to 1 only post-normalisation)
  scoresT_h [T, t] = k_h^T q_h  (K=64, head pair row-packed: A rows 0-63,
        B rows 64-127 of the PE array; outputs share one [128,1024] PSUM
        tile: A in cols 0-511, B in cols 512-1023 — one exp per chunk)
  P = exp(scoresT)  (softmax max-subtraction skipped: scores ~ N(0,1))
  l_h[t] = ones^T P (K=128 M=1 matmul, col-packed pair)
  avT_h [e, t] = v_h^T P (col-packed head pairs), normalise by 1/l via
        K=1 broadcast matmul + reciprocal_approx, fold into bf16 attnout
  y = Wproj attnout + b  -> bf16 out [512, 1024] (cast f32 on host)

Two outer passes over t-halves (th); emission is software-pipelined: per
chunk u emit scores+exp(u) then av/l(u-1), weaving qk/v/proj GEMMs into
chunk slots as PE filler so the exp stream never starves.
"""

import numpy as np
import ml_dtypes

_CACHE = {}

B, C, T = 8, 512, 1024
NH, E = 8, 64


def _build():
    import concourse.tile as tile
    from concourse import bacc, mybir

    F32 = mybir.dt.float32
    BF16 = mybir.dt.bfloat16
    AF = mybir.ActivationFunctionType
    ALU = mybir.AluOpType

    import concourse.bass as bass

    nc = bacc.Bacc()

    x_d = nc.declare_dram_parameter("x", [C, T], BF16, isOutput=False)
    wqkv_d = nc.declare_dram_parameter("wqkv", [C, 3 * C], BF16, isOutput=False)
    wproj_d = nc.declare_dram_parameter("wproj", [C, C], BF16, isOutput=False)
    bqk_d = nc.declare_dram_parameter("bqk", [128, 8], F32, isOutput=False)
    bvp_d = nc.declare_dram_parameter("bvp", [128, 4], F32, isOutput=False)
    bproj_d = nc.declare_dram_parameter("bproj", [128, 4], F32, isOutput=False)
    out_d = nc.declare_dram_parameter("out", [C, T], BF16, isOutput=True)

    with tile.TileContext(nc) as tc:
        with (
            tc.tile_pool(name="persist", bufs=1) as per,
            tc.tile_pool(name="ppool", bufs=2) as pp,
            tc.tile_pool(name="scr", bufs=3) as scr,
            tc.tile_pool(name="dram", bufs=8, space="DRAM") as dr,
            tc.tile_pool(name="psc", bufs=2, space="PSUM") as psc,
            tc.tile_pool(name="pav", bufs=2, space="PSUM") as pav,
            tc.tile_pool(name="pms", bufs=2, space="PSUM") as pms,
        ):
            # ---- ones + PE warmup first (no input deps): release the HAM
            # clock gate while input DMAs land.
            ones_all = per.tile([128, 512], BF16, tag="ones_all")
            nc.vector.memset(ones_all[:], 1.0)
            pswarm = pms.tile([128, 512], F32, tag="ms", name="pswarm")
            for i in range(12):
                nc.tensor.matmul(
                    pswarm[:], lhsT=ones_all[:, 0:128], rhs=ones_all[:],
                    start=True, stop=True,
                )

            # ---- input DMAs. Critical path: x + q-part + k-part of wqkv +
            # bqk gate the first GEMMs — spread across engine queues.
            bqk = per.tile([128, 8], F32, tag="bqk")
            nc.scalar.dma_start(out=bqk[:], in_=bqk_d[:, :])
            xs = []
            for c in range(4):
                xt = per.tile([128, T], BF16, tag=f"x{c}", name=f"x{c}")
                eng = nc.sync if c < 2 else nc.scalar
                eng.dma_start(out=xt[:], in_=x_d[128 * c : 128 * (c + 1), :])
                xs.append(xt)
            wqkv = [
                per.tile([128, 3 * C], BF16, tag=f"wqkv{c}", name=f"wqkv{c}")
                for c in range(4)
            ]
            for c in range(4):  # q columns (gate qk_gemm(0..3))
                nc.gpsimd.dma_start(
                    out=wqkv[c][:, 0:C], in_=wqkv_d[128 * c : 128 * (c + 1), 0:C]
                )
            for c in range(4):  # k columns
                nc.scalar.dma_start(
                    out=wqkv[c][:, C : 2 * C],
                    in_=wqkv_d[128 * c : 128 * (c + 1), C : 2 * C],
                )
            bvp = per.tile([128, 4], F32, tag="bvp")
            nc.gpsimd.dma_start(out=bvp[:], in_=bvp_d[:, :])
            for c in range(4):  # v columns
                nc.sync.dma_start(
                    out=wqkv[c][:, 2 * C : 3 * C],
                    in_=wqkv_d[128 * c : 128 * (c + 1), 2 * C : 3 * C],
                )
            wproj = []
            for c in range(4):
                w = per.tile([128, C], BF16, tag=f"wproj{c}", name=f"wproj{c}")
                nc.gpsimd.dma_start(out=w[:], in_=wproj_d[128 * c : 128 * (c + 1), :])
                wproj.append(w)
            bproj = per.tile([128, 4], F32, tag="bproj")
            nc.gpsimd.dma_start(out=bproj[:], in_=bproj_d[:, :])

            # ---- persistent activations ----
            qk = [per.tile([128, T], BF16, tag=f"qk{o}", name=f"qk{o}") for o in range(8)]
            vT = [per.tile([128, C], BF16, tag=f"vT{t}", name=f"vT{t}") for t in range(8)]
            attnout = [
                per.tile([128, T], BF16, tag=f"ao{p}", name=f"ao{p}") for p in range(4)
            ]
            out_sb = [
                per.tile([128, T], BF16, tag=f"os{o}", name=f"os{o}") for o in range(4)
            ]

            qk_ps = {}

            def qk_gemm_half(o, half):
                # half 0/1 = t-columns [0:512] / [512:1024] of the qk GEMM
                if half == 0:
                    qk_ps[o] = psc.tile([128, T], F32, tag="sc", name=f"psqk{o}")
                ps = qk_ps[o]
                for c in range(4):
                    nc.tensor.matmul(
                        ps[:, 512 * half : 512 * (half + 1)],
                        lhsT=wqkv[c][:, 128 * o : 128 * (o + 1)],
                        rhs=xs[c][:, 512 * half : 512 * (half + 1)],
                        start=(c == 0),
                        stop=(c == 3),
                    )
                if half == 1:
                    if o == 0:  # ScalarE is idle pre-attention; Identity is in
                        # the exp table set (no ACT_TABLE_LOAD switch)
                        nc.scalar.activation(
                            qk[o][:], ps[:], AF.Identity, bias=bqk[:, o : o + 1]
                        )
                    else:
                        nc.vector.tensor_scalar_add(qk[o][:], ps[:], bqk[:, o : o + 1])

            v_ps = {}

            def v_gemm_half(tt, half):
                if half == 0:
                    v_ps[tt] = pms.tile([128, 512], F32, tag="ms", name=f"psv{tt}")
                    for c in range(2):
                        nc.tensor.matmul(
                            v_ps[tt][:],
                            lhsT=xs[c][:, 128 * tt : 128 * (tt + 1)],
                            rhs=wqkv[c][:, 2 * C : 3 * C],
                            start=(c == 0), stop=False,
                        )
                else:
                    ps = v_ps[tt]
                    for c in range(2, 4):
                        nc.tensor.matmul(
                            ps[:],
                            lhsT=xs[c][:, 128 * tt : 128 * (tt + 1)],
                            rhs=wqkv[c][:, 2 * C : 3 * C],
                            start=False, stop=(c == 3),
                        )
                    nc.vector.tensor_copy(vT[tt][:], ps[:])

            def scexp(p, th, k):
                tsl = slice(512 * th, 512 * (th + 1))
                ksl = slice(128 * k, 128 * (k + 1))
                ps = psc.tile([128, T], F32, tag="sc", name=f"pssc{p}_{th}_{k}")
                nc.tensor.matmul(
                    ps[:, 0:512],
                    lhsT=qk[4 + p][0:64, ksl], rhs=qk[p][0:64, tsl],
                    start=True, stop=True, tile_position=(0, 0),
                )
                nc.tensor.matmul(
                    ps[:, 512:1024],
                    lhsT=qk[4 + p][64:128, ksl], rhs=qk[p][64:128, tsl],
                    start=True, stop=True, tile_position=(64, 0),
                )
                P = pp.tile([128, T], BF16, tag=f"P{k}", bufs=3,
                            name=f"P{p}_{th}_{k}")
                nc.scalar.activation(P[:], ps[:], AF.Exp)
                return P

            def av_mms(p, th, k, P, psav, psl):
                hA, hB = 2 * p, 2 * p + 1
                nc.tensor.matmul(
                    psav[0:64, :],
                    lhsT=vT[k][:, 64 * hA : 64 * (hA + 1)], rhs=P[:, 0:512],
                    start=(k == 0), stop=(k == 7), tile_position=(0, 0),
                )
                nc.tensor.matmul(
                    psav[64:128, :],
                    lhsT=vT[k][:, 64 * hB : 64 * (hB + 1)], rhs=P[:, 512:1024],
                    start=(k == 0), stop=(k == 7), tile_position=(0, 64),
                )
                nc.tensor.matmul(
                    psl[0:1, :],
                    lhsT=ones_all[:, 0:1], rhs=P[:, 0:512],
                    start=(k == 0), stop=(k == 7), tile_position=(0, 0),
                )
                nc.tensor.matmul(
                    psl[32:33, :],
                    lhsT=ones_all[:, 0:1], rhs=P[:, 512:1024],
                    start=(k == 0), stop=(k == 7), tile_position=(0, 32),
                )

            def finalize(p, th, psav, psl):
                tsl = slice(512 * th, 512 * (th + 1))
                lb = scr.tile([128, 512], BF16, tag="lb", name=f"lb{p}_{th}")
                nc.vector.tensor_copy(lb[:], psl[:])
                psbc = pms.tile([128, 512], F32, tag="ms", name=f"psbc{p}_{th}")
                nc.tensor.matmul(
                    psbc[0:64, :], lhsT=ones_all[0:1, 0:64], rhs=lb[0:1, :],
                    start=True, stop=True, tile_position=(0, 0),
                )
                nc.tensor.matmul(
                    psbc[64:128, :], lhsT=ones_all[32:33, 0:64], rhs=lb[32:33, :],
                    start=True, stop=True, tile_position=(32, 64),
                )
                bc = scr.tile([128, 512], F32, tag="bc", name=f"bc{p}_{th}")
                nc.vector.reciprocal_approx_fast(bc[:], psbc[:])
                tmpn = scr.tile([128, 512], F32, tag="tmpn", name=f"tn{p}_{th}")
                nc.vector.tensor_mul(tmpn[:], psav[:], bc[:])
                nc.vector.tensor_scalar_add(
                    attnout[p][:, tsl], tmpn[:], bvp[:, p : p + 1]
                )

            def proj(o, th):
                tsl = slice(512 * th, 512 * (th + 1))
                ps = pms.tile([128, 512], F32, tag="ms", name=f"pspr{o}_{th}")
                for c in range(4):
                    nc.tensor.matmul(
                        ps[:],
                        lhsT=wproj[c][:, 128 * o : 128 * (o + 1)],
                        rhs=attnout[c][:, tsl],
                        start=(c == 0),
                        stop=(c == 3),
                    )
                nc.vector.tensor_scalar_add(out_sb[o][:, tsl], ps[:], bproj[:, o : o + 1])
                eng = nc.sync if o % 2 == 0 else nc.gpsimd
                eng.dma_start(
                    out=out_d[128 * o : 128 * (o + 1), tsl], in_=out_sb[o][:, tsl]
                )

            # ---- schedule: two passes over t-halves, software-pipelined;
            # av/l batched two chunks behind the score/exp stream.
            pending = []
            fin_q = []
            carry = {}

            def emit_avl(limit=1):
                nonlocal pending
                if len(pending) <= limit:
                    return
                todo, pending = pending[:-limit] if limit else pending, \
                    pending[-limit:] if limit else []
                for u in todo:
                    av_mms(*u)

            quarters = [
                (0, (0, 1)), (1, (0, 1)), (0, (2, 3)), (1, (2, 3)),
            ]
            for qi, (th, pairs) in enumerate(quarters):
                fill = {}
                if qi == 0:
                    # v GEMM halves through pair 0; qk1/qk5 late in pair 0
                    fill[(0, 0)] = [lambda: v_gemm_half(0, 0), lambda: v_gemm_half(0, 1),
                                    lambda: v_gemm_half(1, 0)]
                    fill[(0, 1)] = [lambda: v_gemm_half(1, 1), lambda: v_gemm_half(2, 0)]
                    fill[(0, 2)] = [lambda: v_gemm_half(2, 1), lambda: v_gemm_half(3, 0)]
                    fill[(0, 3)] = [lambda: v_gemm_half(3, 1), lambda: v_gemm_half(4, 0)]
                    fill[(0, 4)] = [lambda: v_gemm_half(4, 1), lambda: v_gemm_half(5, 0),
                                    lambda: qk_gemm_half(5, 0)]
                    fill[(0, 5)] = [lambda: v_gemm_half(5, 1), lambda: v_gemm_half(6, 0),
                                    lambda: qk_gemm_half(5, 1)]
                    fill[(0, 6)] = [lambda: v_gemm_half(6, 1), lambda: v_gemm_half(7, 0),
                                    lambda: qk_gemm_half(1, 0)]
                    fill[(0, 7)] = [lambda: v_gemm_half(7, 1), lambda: qk_gemm_half(1, 1)]
                elif qi == 1:
                    fill[(0, 1)] = [lambda: qk_gemm_half(2, 0)]
                    fill[(0, 4)] = [lambda: qk_gemm_half(2, 1)]
                    fill[(1, 1)] = [lambda: qk_gemm_half(6, 0)]
                    fill[(1, 4)] = [lambda: qk_gemm_half(6, 1)]
                elif qi == 2:
                    fill[(2, 0)] = [lambda: qk_gemm_half(3, 0)]
                    fill[(2, 2)] = [lambda: qk_gemm_half(3, 1)]
                    fill[(2, 4)] = [lambda: qk_gemm_half(7, 0)]
                    fill[(2, 6)] = [lambda: qk_gemm_half(7, 1)]
                else:
                    fill[(2, 2)] = [lambda: proj(0, 0)]
                    fill[(2, 4)] = [lambda: proj(1, 0)]
                    fill[(2, 6)] = [lambda: proj(2, 0)]
                    fill[(3, 2)] = [lambda: proj(3, 0)]
                if qi == 0:
                    qk_gemm_half(0, 0)
                    qk_gemm_half(0, 1)
                    qk_gemm_half(4, 0)
                    qk_gemm_half(4, 1)
                for p in pairs:
                    psav = pav.tile([128, 512], F32, tag="av", name=f"psav{p}_{th}")
                    psl = pms.tile([128, 512], F32, tag="ms", name=f"psl{p}_{th}")
                    last = (qi == 3 and p == 3)
                    for k in range(8):
                        P = scexp(p, th, k)
                        if k == 0:
                            emit_avl(limit=2)
                        elif k == 1:
                            emit_avl(limit=0)
                        elif last:
                            emit_avl(limit=1)
                        elif k % 2 == 0:
                            emit_avl(limit=2)
                        for f in fill.get((p, k), []):
                            f()
                        while fin_q:
                            fin_q.pop(0)()
                        pending.append((p, th, k, P, psav, psl))
                        if k == 0 and (p != 0 or th != 0):
                            pp_, pth, pav_, pl_ = carry["prev"]
                            fin_q.append(
                                lambda a=pp_, b=pth, c=pav_, d=pl_: finalize(a, b, c, d)
                            )
                    carry["prev"] = (p, th, psav, psl)
            emit_avl(limit=0)
            finalize(*carry["prev"])
            for o in range(4):
                proj(o, 1)

    nc.compile()
    return nc


def _get_nc():
    if "nc" not in _CACHE:
        _CACHE["nc"] = _build()
    return _CACHE["nc"]


def kernel(x, qkv_w, qkv_b, proj_w, proj_b, _trace=False):
    from concourse.bass_utils import run_bass_kernel_spmd

    nc = _get_nc()

    bf16 = ml_dtypes.bfloat16
    b, c, h, w = x.shape
    xf = np.asarray(x, dtype=np.float32).reshape(b, c, h * w)
    qkv_b = np.asarray(qkv_b, dtype=np.float32)
    qkv_w = np.asarray(qkv_w, dtype=np.float32)
    # fold the 1/sqrt(e)=1/8 softmax scale into Wq / bq on host
    qkv_w = np.concatenate([qkv_w[:512] * 0.125, qkv_w[512:]], axis=0)
    bq = np.concatenate([qkv_b[:512] * 0.125, qkv_b[512:1024]])
    wqkvT = np.ascontiguousarray(qkv_w.T).astype(bf16)
    wprojT = np.ascontiguousarray(np.asarray(proj_w, np.float32).T).astype(bf16)
    bqk = np.ascontiguousarray(bq.reshape(8, 128).T)
    bvp = np.ascontiguousarray(qkv_b[1024:1536].reshape(4, 128).T)
    bproj = np.ascontiguousarray(np.asarray(proj_b, np.float32).reshape(4, 128).T)

    in_maps = [
        dict(
            x=np.ascontiguousarray(xf[i]).astype(bf16),
            wqkv=wqkvT,
            wproj=wprojT,
            bqk=bqk,
            bvp=bvp,
            bproj=bproj,
        )
        for i in range(b)
    ]
    res = run_bass_kernel_spmd(nc, in_maps, core_ids=list(range(8)), trace=_trace)
    out = np.stack([res.results[i]["out"].astype(np.float32) for i in range(b)])
    out = out.reshape(b, c, h, w)
    if _trace:
        _CACHE["last_result"] = res
    return out
